# revision 1
# baseline (speedup 1.0000x reference)
"""Trainium2 Bass kernel: single-head causal attention with low-rank V.

Math (per batch b):
  Q = i@q, K = i@k                  [4096, 64]
  S = Q K^T  (causal mask, /8)      [4096, 4096]
  A = softmax(S)
  out = A @ ((i@v1) @ v2)           [4096, 512]
     = ((A @ (i@v1)) / l) @ v2      (reassociated low-rank form)

Sharding: 8 cores = 4 batches x 2 halves. Within a batch, core half h
owns query tiles g = 2t+h (t=0..15, 128 rows each) -> balanced causal
work. All cores run one SPMD program; per-core differences live in the
data (pre-transposed i^T, own-query i^T, causal masks).

Device pipeline per core:
  P1: QT = wq^T @ iTq, KT = wk^T @ iT   (PE, score dtype)
  P2: pvT = v1^T @ iT -> transpose -> pv [sk,65] fp16 (ones col for l)
  P3: per query tile t (chunks of 512 keys, groups of 2 chunks):
      pass A: S = QT_t^T K + mask (PE) -> row max (DVE)
      pass B: recompute S + mask -> E = exp((S-m)/8) fp16 (ACT)
              -> E^T (PE transpose) -> O[128,65] += E @ pv (PE)
      finalize: l=O[:,64], O/l -> fp16 -> O^T (PE) -> out = O^T^T @ v2
"""

import sys

if "/opt/trn_rl_repo" not in sys.path:
    sys.path.insert(0, "/opt/trn_rl_repo")

from contextlib import ExitStack

import numpy as np

import concourse.bass as bass
import concourse.tile as tile
from concourse import bacc
from concourse import mybir
from concourse.bass_utils import run_bass_kernel_spmd

B, LN, IDM, HDM = 4, 4096, 512, 64
P = 128          # partitions / tile rows
NT = 16          # query tiles per core
NKT = 32         # key tiles per batch
SC = 512         # key chunk (1 psum bank of fp32)
MASK_NEG = -60000.0  # "-inf" in fp16-representable units (pre-scale)

# chunks of 512 keys needed by local query tile t (same for both halves)
CHUNKS = [(t + 2) // 2 for t in range(NT)]   # 1,1,2,2,...,8,8


def _groups(cc):
    """Split cc chunks into groups of <=2 chunks: [(start_chunk, n_chunks)]"""
    out = []
    c = 0
    while c < cc:
        n = min(2, cc - c)
        out.append((c, n))
        c += n
    return out


def build_nc(score_dt):
    f32 = mybir.dt.float32
    f16 = mybir.dt.float16
    nc = bacc.Bacc()

    iT = nc.dram_tensor("iT", [IDM, LN], score_dt, kind="ExternalInput")
    iTq = nc.dram_tensor("iTq", [IDM, NT * P], score_dt, kind="ExternalInput")
    wq = nc.dram_tensor("wq", [IDM, HDM], score_dt, kind="ExternalInput")
    wk = nc.dram_tensor("wk", [IDM, HDM], score_dt, kind="ExternalInput")
    v1s = nc.dram_tensor("v1s", [IDM, HDM], score_dt, kind="ExternalInput")
    v2h = nc.dram_tensor("v2h", [HDM, IDM], f16, kind="ExternalInput")
    masks = nc.dram_tensor("masks", [NT, P, SC], f16, kind="ExternalInput")
    out = nc.dram_tensor("out", [NT, P, IDM], f32, kind="ExternalOutput")

    with tile.TileContext(nc) as tc, ExitStack() as ctx:
        singles = ctx.enter_context(tc.tile_pool(name="singles", bufs=1))

        # ---- resident SBUF inputs ----
        iT_sb = singles.tile([P, 4, LN], score_dt)
        nc.sync.dma_start(out=iT_sb, in_=iT.rearrange("(c p) n -> p c n", p=P))
        iTq_sb = singles.tile([P, 4, NT * P], score_dt)
        nc.sync.dma_start(out=iTq_sb, in_=iTq.rearrange("(c p) n -> p c n", p=P))
        wq_sb = singles.tile([P, 4, HDM], score_dt)
        nc.sync.dma_start(out=wq_sb, in_=wq.rearrange("(c p) h -> p c h", p=P))
        wk_sb = singles.tile([P, 4, HDM], score_dt)
        nc.sync.dma_start(out=wk_sb, in_=wk.rearrange("(c p) h -> p c h", p=P))
        v1_sb = singles.tile([P, 4, HDM], score_dt)
        nc.sync.dma_start(out=v1_sb, in_=v1s.rearrange("(c p) h -> p c h", p=P))
        v2_sb = singles.tile([HDM, IDM], f16)
        nc.sync.dma_start(out=v2_sb, in_=v2h[:, :])
        mask_sb = singles.tile([P, NT, SC], f16)
        nc.sync.dma_start(out=mask_sb, in_=masks.rearrange("t p m -> p t m"))

        # fp16 identity for PE transposes and mask-add matmuls
        id16 = singles.tile([P, P], f16)
        from concourse.masks import make_identity
        make_identity(nc, id16)

        QT_sb = singles.tile([HDM, NT * P], score_dt)   # Q^T of own rows
        KT_sb = singles.tile([HDM, LN], score_dt)       # K^T full batch
        pv_sb = singles.tile([P, NKT, HDM + 1], f16)    # pv rows + ones col

        # ---- phase 1: projections ----
        with tc.tile_pool(name="proj_ps", bufs=2, space="PSUM") as proj_ps:
            for j in range(NT * P // SC):          # Q^T, 4 slices of 512
                ps = proj_ps.tile([HDM, SC], f32, tag="proj")
                for c in range(4):
                    nc.tensor.matmul(
                        ps, lhsT=wq_sb[:, c, :],
                        rhs=iTq_sb[:, c, j * SC:(j + 1) * SC],
                        start=(c == 0), stop=(c == 3))
                nc.scalar.copy(out=QT_sb[:, j * SC:(j + 1) * SC], in_=ps)
            for j in range(LN // SC):              # K^T, 8 slices of 512
                ps = proj_ps.tile([HDM, SC], f32, tag="proj")
                for c in range(4):
                    nc.tensor.matmul(
                        ps, lhsT=wk_sb[:, c, :],
                        rhs=iT_sb[:, c, j * SC:(j + 1) * SC],
                        start=(c == 0), stop=(c == 3))
                nc.scalar.copy(out=KT_sb[:, j * SC:(j + 1) * SC], in_=ps)

            # ---- phase 2: pv = i @ v1 (transposed back), plus ones column
            pvt_sb = singles.tile([HDM, LN], f16)
            for j in range(LN // SC):
                ps = proj_ps.tile([HDM, SC], f32, tag="proj")
                for c in range(4):
                    nc.tensor.matmul(
                        ps, lhsT=v1_sb[:, c, :],
                        rhs=iT_sb[:, c, j * SC:(j + 1) * SC],
                        start=(c == 0), stop=(c == 3))
                nc.scalar.copy(out=pvt_sb[:, j * SC:(j + 1) * SC], in_=ps)
            for kt in range(NKT):                  # transpose [64,128]->[128,64]
                pb = proj_ps.tile([P, HDM], f16, tag="pvblk")
                nc.tensor.transpose(
                    pb, in_=pvt_sb[:, kt * P:(kt + 1) * P], identity=id16[:HDM, :HDM])
                nc.vector.tensor_copy(pv_sb[:, kt, 0:HDM], pb)
            nc.vector.memset(pv_sb[:, :, HDM:HDM + 1], 1.0)

        # ---- phase 3: attention ----
        spool = ctx.enter_context(tc.tile_pool(name="spool", bufs=2, space="PSUM"))
        etpool = ctx.enter_context(tc.tile_pool(name="etpool", bufs=1, space="PSUM"))
        opool = ctx.enter_context(tc.tile_pool(name="opool", bufs=1, space="PSUM"))
        fpool = ctx.enter_context(tc.tile_pool(name="fpool", bufs=1, space="PSUM"))
        sb = ctx.enter_context(tc.tile_pool(name="sbwork", bufs=3))
        stat = ctx.enter_context(tc.tile_pool(name="stat", bufs=2))

        for t in range(NT):
            cc = CHUNKS[t]
            qslice = QT_sb[:, t * P:(t + 1) * P]
            grps = _groups(cc)

            def s_group(g0, gn, tag):
                """matmul S for chunks [g0, g0+gn) of qtile t into one psum tile."""
                ps = spool.tile([P, gn * SC], f32, tag=tag)
                for ci in range(gn):
                    c = g0 + ci
                    nc.tensor.matmul(
                        ps[:, ci * SC:(ci + 1) * SC], lhsT=qslice,
                        rhs=KT_sb[:, c * SC:(c + 1) * SC],
                        start=True, stop=(c != cc - 1))
                    if c == cc - 1:   # add causal/pad mask via identity matmul
                        nc.tensor.matmul(
                            ps[:, ci * SC:(ci + 1) * SC], lhsT=id16,
                            rhs=mask_sb[:, t, :], start=False, stop=True,
                            skip_group_check=True)
                return ps

            # pass A: row max
            mxp = stat.tile([P, 4], f32, tag="mxp")
            nc.vector.memset(mxp, -3.0e38)
            for gi, (g0, gn) in enumerate(grps):
                ps = s_group(g0, gn, "s")
                nc.vector.reduce_max(
                    out=mxp[:, gi:gi + 1], in_=ps, axis=mybir.AxisListType.X)
            mneg = stat.tile([P, 1], f32, tag="mneg")
            nc.vector.tensor_scalar_mul(mneg, mxp[:, 0:1], 1.0)  # placeholder
            m = stat.tile([P, 1], f32, tag="m")
            nc.vector.reduce_max(out=m, in_=mxp, axis=mybir.AxisListType.X)
            nc.vector.tensor_scalar_mul(mneg, m, -0.125)

            # pass B: recompute S, exp, E^T, O accumulation
            o_ps = opool.tile([P, HDM + 1], f32, tag="o")
            for gi, (g0, gn) in enumerate(grps):
                ps = s_group(g0, gn, "s")
                e_sb = sb.tile([P, 2 * SC], f16, tag="e")
                nc.scalar.activation(
                    out=e_sb[:, :gn * SC], in_=ps,
                    func=mybir.ActivationFunctionType.Exp,
                    bias=mneg, scale=0.125)
                for ci in range(gn):
                    c = g0 + ci
                    et_ps = etpool.tile([P, 4, P], f16, tag="et")
                    for u in range(4):
                        nc.tensor.transpose(
                            et_ps[:, u, :],
                            in_=e_sb[:, ci * SC + u * P: ci * SC + (u + 1) * P],
                            identity=id16)
                    et_sb = sb.tile([P, 4, P], f16, tag="etsb")
                    nc.vector.tensor_copy(et_sb, et_ps)
                    for u in range(4):
                        kt = 4 * c + u
                        nc.tensor.matmul(
                            o_ps, lhsT=et_sb[:, u, :], rhs=pv_sb[:, kt, :],
                            start=(kt == 0), stop=(kt == 4 * cc - 1))

            # finalize qtile
            linv = stat.tile([P, 1], f32, tag="linv")
            nc.vector.reciprocal(linv, o_ps[:, HDM:HDM + 1])
            on_sb = sb.tile([P, HDM], f16, tag="on")
            nc.vector.tensor_scalar_mul(on_sb, o_ps[:, 0:HDM], linv)
            ot_ps = etpool.tile([HDM, P], f16, tag="ot")
            nc.tensor.transpose(ot_ps, in_=on_sb, identity=id16)
            ot_sb = sb.tile([HDM, P], f16, tag="otsb")
            nc.vector.tensor_copy(ot_sb, ot_ps)
            f_ps = fpool.tile([P, IDM], f32, tag="f")
            nc.tensor.matmul(f_ps, lhsT=ot_sb, rhs=v2_sb, start=True, stop=True)
            f_sb = sb.tile([P, IDM], f32, tag="fsb")
            nc.scalar.copy(out=f_sb, in_=f_ps)
            nc.sync.dma_start(out=out[t], in_=f_sb)

    nc.finalize()
    return nc


def make_core_inputs(inputs, score_np=np.float32):
    """Build the 8 per-core in_maps from full inputs (host-side shard prep)."""
    i = np.asarray(inputs["i"], dtype=np.float32)
    q = np.asarray(inputs["q"], dtype=np.float32)
    k = np.asarray(inputs["k"], dtype=np.float32)
    v1 = np.asarray(inputs["v1"], dtype=np.float32)
    v2 = np.asarray(inputs["v2"], dtype=np.float32)

    v2h = np.ascontiguousarray(v2.astype(np.float16))
    in_maps = []
    for core in range(8):
        b, h = core // 2, core % 2
        ib = i[b]                                    # [4096, 512]
        qrows = np.concatenate(
            [np.arange(128 * (2 * t + h), 128 * (2 * t + h) + 128)
             for t in range(NT)])
        iT = np.ascontiguousarray(ib.T)              # [512, 4096]
        iTq = np.ascontiguousarray(ib[qrows].T)      # [512, 2048]
        # masks: per qtile t, last 512-key chunk [128, 512]
        mk = np.zeros((NT, P, SC), dtype=np.float16)
        for t in range(NT):
            g = 2 * t + h
            c_last = CHUNKS[t] - 1
            key0 = c_last * SC                       # global key col of chunk
            keys = key0 + np.arange(SC)[None, :]     # [1, 512]
            rows = g * P + np.arange(P)[:, None]     # [128, 1]
            mk[t] = np.where(keys <= rows, 0.0, MASK_NEG).astype(np.float16)
        in_maps.append({
            "iT": iT, "iTq": iTq,
            "wq": np.ascontiguousarray(q), "wk": np.ascontiguousarray(k),
            "v1s": np.ascontiguousarray(v1), "v2h": v2h,
            "masks": mk,
        })
    return in_maps




def build_nc_v2(score_dt):
    """v2: permuted-column iT (uniform Q slices), packed QK projection,
    single-pass online-rescale flash (no score recompute)."""
    f32 = mybir.dt.float32
    f16 = mybir.dt.float16
    nc = bacc.Bacc()

    iT = nc.dram_tensor("iT", [IDM, LN], score_dt, kind="ExternalInput")
    wqk = nc.dram_tensor("wqk", [IDM, P], score_dt, kind="ExternalInput")
    iTh = nc.dram_tensor("iTh", [IDM, LN], f16, kind="ExternalInput")
    v1h = nc.dram_tensor("v1h", [IDM, HDM], f16, kind="ExternalInput")
    v2h = nc.dram_tensor("v2h", [HDM, IDM], f16, kind="ExternalInput")
    masks = nc.dram_tensor("masks", [NT, P, SC], f16, kind="ExternalInput")
    out = nc.dram_tensor("out", [NT, P, IDM], f32, kind="ExternalOutput")

    with tile.TileContext(nc) as tc, ExitStack() as ctx:
        singles = ctx.enter_context(tc.tile_pool(name="singles", bufs=1))

        iT_sb = singles.tile([P, 4, LN], score_dt)
        nc.sync.dma_start(out=iT_sb, in_=iT.rearrange("(c p) n -> p c n", p=P))
        iTh_sb = singles.tile([P, 4, LN], f16)
        nc.sync.dma_start(out=iTh_sb, in_=iTh.rearrange("(c p) n -> p c n", p=P))
        wqk_sb = singles.tile([P, 4, P], score_dt)
        nc.sync.dma_start(out=wqk_sb, in_=wqk.rearrange("(c p) h -> p c h", p=P))
        v1_sb = singles.tile([P, 4, HDM], f16)
        nc.sync.dma_start(out=v1_sb, in_=v1h.rearrange("(c p) h -> p c h", p=P))
        v2_sb = singles.tile([HDM, IDM], f16)
        nc.sync.dma_start(out=v2_sb, in_=v2h[:, :])
        mask_sb = singles.tile([P, NT, SC], f16)
        nc.sync.dma_start(out=mask_sb, in_=masks.rearrange("t p m -> p t m"))

        id16 = singles.tile([P, P], f16)
        from concourse.masks import make_identity
        make_identity(nc, id16)

        QT_sb = singles.tile([HDM, LN], score_dt)
        KT_sb = singles.tile([HDM, LN], score_dt)
        pv_sb = singles.tile([P, NKT, HDM + 1], f16)

        with tc.tile_pool(name="proj_ps", bufs=2, space="PSUM") as proj_ps:
            for j in range(LN // SC):              # packed Q|K, 8 slices
                ps = proj_ps.tile([P, SC], f32, tag="proj")
                for c in range(4):
                    nc.tensor.matmul(
                        ps, lhsT=wqk_sb[:, c, :],
                        rhs=iT_sb[:, c, j * SC:(j + 1) * SC],
                        start=(c == 0), stop=(c == 3))
                nc.scalar.copy(out=QT_sb[:, j * SC:(j + 1) * SC], in_=ps[0:HDM, :])
                nc.scalar.copy(out=KT_sb[:, j * SC:(j + 1) * SC], in_=ps[HDM:P, :])

            pvt_sb = singles.tile([HDM, LN], f16)
            for j in range(LN // SC):
                ps = proj_ps.tile([HDM, SC], f32, tag="pvproj")
                for c in range(4):
                    nc.tensor.matmul(
                        ps, lhsT=v1_sb[:, c, :],
                        rhs=iTh_sb[:, c, j * SC:(j + 1) * SC],
                        start=(c == 0), stop=(c == 3))
                nc.scalar.copy(out=pvt_sb[:, j * SC:(j + 1) * SC], in_=ps)
            for kt in range(NKT):
                pb = proj_ps.tile([P, HDM], f16, tag="pvblk")
                nc.tensor.transpose(
                    pb, in_=pvt_sb[:, kt * P:(kt + 1) * P], identity=id16[:HDM, :HDM])
                nc.vector.tensor_copy(pv_sb[:, kt, 0:HDM], pb)
            nc.vector.memset(pv_sb[:, :, HDM:HDM + 1], 1.0)

        spool = ctx.enter_context(tc.tile_pool(name="spool", bufs=2, space="PSUM"))
        etpool = ctx.enter_context(tc.tile_pool(name="etpool", bufs=1, space="PSUM"))
        opool = ctx.enter_context(tc.tile_pool(name="opool", bufs=1, space="PSUM"))
        fpool = ctx.enter_context(tc.tile_pool(name="fpool", bufs=1, space="PSUM"))
        sb = ctx.enter_context(tc.tile_pool(name="sbwork", bufs=3))
        stat = ctx.enter_context(tc.tile_pool(name="stat", bufs=2))

        for t in range(NT):
            cc = CHUNKS[t]
            qslice = QT_sb[:, 2 * t * P:(2 * t + 1) * P]
            grps = _groups(cc)
            o_ps = opool.tile([P, HDM + 1], f32, tag="o")
            m_run = None

            for gi, (g0, gn) in enumerate(grps):
                ps = spool.tile([P, gn * SC], f32, tag="s")
                for ci in range(gn):
                    c = g0 + ci
                    nc.tensor.matmul(
                        ps[:, ci * SC:(ci + 1) * SC], lhsT=qslice,
                        rhs=KT_sb[:, c * SC:(c + 1) * SC],
                        start=True, stop=(c != cc - 1))
                    if c == cc - 1:
                        nc.tensor.matmul(
                            ps[:, ci * SC:(ci + 1) * SC], lhsT=id16,
                            rhs=mask_sb[:, t, :], start=False, stop=True,
                            skip_group_check=True)

                mx = stat.tile([P, 1], f32, tag="mx")
                nc.vector.reduce_max(out=mx, in_=ps, axis=mybir.AxisListType.X)
                if m_run is None:
                    m_new = mx
                else:
                    m_new = stat.tile([P, 1], f32, tag="mnew")
                    nc.vector.tensor_max(m_new, m_run, mx)
                    dm = stat.tile([P, 1], f32, tag="dm")
                    nc.vector.tensor_sub(dm, m_run, m_new)
                    al = stat.tile([P, 1], f32, tag="al")
                    nc.scalar.activation(
                        out=al, in_=dm,
                        func=mybir.ActivationFunctionType.Exp, scale=0.125)
                    nc.vector.tensor_scalar_mul(o_ps, o_ps, al)
                mneg = stat.tile([P, 1], f32, tag="mneg")
                nc.vector.tensor_scalar_mul(mneg, m_new, -0.125)
                m_run = m_new

                e_sb = sb.tile([P, 2 * SC], f16, tag="e")
                nc.scalar.activation(
                    out=e_sb[:, :gn * SC], in_=ps,
                    func=mybir.ActivationFunctionType.Exp,
                    bias=mneg, scale=0.125)
                for ci in range(gn):
                    c = g0 + ci
                    et_ps = etpool.tile([P, 4, P], f16, tag="et")
                    for u in range(4):
                        nc.tensor.transpose(
                            et_ps[:, u, :],
                            in_=e_sb[:, ci * SC + u * P: ci * SC + (u + 1) * P],
                            identity=id16)
                    et_sb = sb.tile([P, 4, P], f16, tag="etsb")
                    nc.vector.tensor_copy(et_sb, et_ps)
                    for u in range(4):
                        kt = 4 * c + u
                        nc.tensor.matmul(
                            o_ps, lhsT=et_sb[:, u, :], rhs=pv_sb[:, kt, :],
                            start=(kt == 0), stop=(kt == 4 * cc - 1),
                            skip_group_check=True)

            linv = stat.tile([P, 1], f32, tag="linv")
            nc.vector.reciprocal(linv, o_ps[:, HDM:HDM + 1])
            on_sb = sb.tile([P, HDM], f16, tag="on")
            nc.vector.tensor_scalar_mul(on_sb, o_ps[:, 0:HDM], linv)
            ot_ps = etpool.tile([HDM, P], f16, tag="ot")
            nc.tensor.transpose(ot_ps, in_=on_sb, identity=id16)
            ot_sb = sb.tile([HDM, P], f16, tag="otsb")
            nc.vector.tensor_copy(ot_sb, ot_ps)
            f_ps = fpool.tile([P, IDM], f32, tag="f")
            nc.tensor.matmul(f_ps, lhsT=ot_sb, rhs=v2_sb, start=True, stop=True)
            f_sb = sb.tile([P, IDM], f32, tag="fsb")
            nc.scalar.copy(out=f_sb, in_=f_ps)
            nc.sync.dma_start(out=out[t], in_=f_sb)

    nc.finalize()
    return nc


def make_core_inputs_v2(inputs):
    i = np.asarray(inputs["i"], dtype=np.float32)
    q = np.asarray(inputs["q"], dtype=np.float32)
    k = np.asarray(inputs["k"], dtype=np.float32)
    v1 = np.asarray(inputs["v1"], dtype=np.float32)
    v2 = np.asarray(inputs["v2"], dtype=np.float32)
    v2h = np.ascontiguousarray(v2.astype(np.float16))
    wqk = np.ascontiguousarray(np.concatenate([q, k], axis=1))
    in_maps = []
    for core in range(8):
        b, h = core // 2, core % 2
        perm_blocks = []
        for j in range(NT):
            perm_blocks += [2 * j + h, 2 * j + 1 - h]
        cols = np.concatenate(
            [np.arange(128 * g, 128 * g + 128) for g in perm_blocks])
        iTp = np.ascontiguousarray(i[b].T[:, cols])      # [512, 4096] permuted
        pb_arr = np.asarray(perm_blocks)
        mk = np.zeros((NT, P, SC), dtype=np.float16)
        for t in range(NT):
            g = 2 * t + h
            key0 = (CHUNKS[t] - 1) * SC
            pc = key0 + np.arange(SC)
            gk = 128 * pb_arr[pc // 128] + pc % 128       # global key index
            rows = g * P + np.arange(P)[:, None]
            mk[t] = np.where(gk[None, :] <= rows, 0.0, MASK_NEG).astype(np.float16)
        in_maps.append({
            "iT": iTp, "iTh": iTp.astype(np.float16), "wqk": wqk,
            "v1h": np.ascontiguousarray(v1.astype(np.float16)),
            "v2h": v2h, "masks": mk,
        })
    return in_maps


_NC_CACHE = {}


def run(inputs, score_dt_name="float32r", variant="v1", **spmd_kwargs):
    score_dt = getattr(mybir.dt, score_dt_name)
    key = (variant, score_dt_name)
    if key not in _NC_CACHE:
        _NC_CACHE[key] = (build_nc_v2 if variant == "v2" else build_nc)(score_dt)
    nc = _NC_CACHE[key]
    in_maps = (make_core_inputs_v2 if variant == "v2" else make_core_inputs)(inputs)
    res = run_bass_kernel_spmd(nc, in_maps, core_ids=list(range(8)), **spmd_kwargs)
    full = np.zeros((B, LN, IDM), dtype=np.float32)
    for core in range(8):
        b, h = core // 2, core % 2
        o = res.results[core]["out"]                 # [16, 128, 512]
        for t in range(NT):
            g = 2 * t + h
            full[b, g * P:(g + 1) * P] = o[t]
    return full, res


def kernel(**inputs):
    full, _ = run(inputs, score_dt_name="float32", variant="v2")
    return full



# revision 25
# speedup vs baseline: 1.4357x; 1.4357x over previous
"""Trainium2 Bass kernel: single-head causal attention with low-rank V.

Math (per batch b):
  Q = i@q, K = i@k                  [4096, 64]
  S = Q K^T  (causal mask, /8)      [4096, 4096]
  A = softmax(S)
  out = A @ ((i@v1) @ v2) = ((A @ (i@v1)) / l) @ v2   (low-rank reassociation)

Sharding: 8 cores = 4 batches x 2 halves. Core half h owns query tiles
g = 2t+h (t=0..15). One SPMD program; per-core differences are data only
(column-permuted i^T so own tiles sit at even block positions, plus two
[128,128] mask blocks and a [128,2] window tensor).

v3 design (all matmuls fp32r at >=256 free / f16):
  P1: packed QK^T projection (wq|wk), pv = (i@v1) via pvT + PE transpose
  P2 (pass A, row-major): S chunks -> row max m via DVE tensor_mask_reduce
      (causal windows are data), -m written into row 64 of Q^T
  P3 (pass B, transposed): S'^T[k,q] = [K^T;1]^T-style 65-row contraction
      = K^T q - m  (m folded into matmul), exp on ACT -> E^T f16 in SBUF
      directly the lhsT of O accumulation (no PE transposes of E, no DVE
      copy-backs), O[q,65] += E^T^T @ [pv|1] gives both O and l.
      Finalize: O/l -> f16 -> transpose -> @ v2 -> out.
"""

import sys

if "/opt/trn_rl_repo" not in sys.path:
    sys.path.insert(0, "/opt/trn_rl_repo")

from contextlib import ExitStack

import numpy as np

import concourse.bass as bass
import concourse.tile as tile
from concourse import bacc
from concourse import mybir
from concourse.bass_utils import run_bass_kernel_spmd

B, LN, IDM, HDM = 4, 4096, 512, 64
P = 128          # partitions / tile rows
NT = 16          # query tiles per core
NKT = 32         # key (pos-)tiles per batch
SC = 512         # key chunk (1 psum bank of fp32)
MASK_NEG = -60000.0  # "-inf" in f16-representable units (pre-scale)


def build_nc_v3():
    """v4: fp32 projections (exact Q,K); scores via compensated bf16x2
    matmuls (m1 = [Khi;1]^T[Qhi;-m], m2 = [Klo;Khi]^T[Qhi;Qlo]); pass A
    (row max) in plain bf16-hi; pv path in fp32r via bitcast."""
    f32 = mybir.dt.float32
    f16 = mybir.dt.float16
    bf16 = mybir.dt.bfloat16
    f32r = mybir.dt.float32r
    nc = bacc.Bacc()

    iT = nc.dram_tensor("iT", [IDM, LN], f32, kind="ExternalInput")
    wqk = nc.dram_tensor("wqk", [IDM, P], f32, kind="ExternalInput")
    ih = nc.dram_tensor("ih", [IDM, LN], bf16, kind="ExternalInput")
    v1s = nc.dram_tensor("v1s", [IDM, HDM], bf16, kind="ExternalInput")
    v2h = nc.dram_tensor("v2h", [HDM, IDM], f16, kind="ExternalInput")
    maskd = nc.dram_tensor("maskd", [3, P, P], f16, kind="ExternalInput")
    out = nc.dram_tensor("out", [NT, P, IDM], f32, kind="ExternalOutput")

    with tile.TileContext(nc) as tc, ExitStack() as ctx:
        singles = ctx.enter_context(tc.tile_pool(name="singles", bufs=1))

        # ---- small inputs first so projections can start with iT slice 0
        wqk_sb = singles.tile([P, 4, P], f32)
        nc.sync.dma_start(out=wqk_sb, in_=wqk.rearrange("(c p) h -> p c h", p=P))
        v1_sb = singles.tile([P, 4, HDM], bf16)
        nc.sync.dma_start(out=v1_sb, in_=v1s.rearrange("(c p) h -> p c h", p=P))
        v2_sb = singles.tile([HDM, IDM], f16)
        nc.sync.dma_start(out=v2_sb, in_=v2h[:, :])
        maskd_sb = singles.tile([P, 3, P], f16)
        nc.sync.dma_start(out=maskd_sb, in_=maskd.rearrange("m p q -> p m q"))
        iT_sb = singles.tile([P, 4, LN], f32)
        iT_r = iT.rearrange("(c p) n -> p c n", p=P)
        ih_sb = singles.tile([P, 4, LN], bf16)
        ih_r = ih.rearrange("(c p) n -> p c n", p=P)
        for j in range(LN // SC):
            nc.sync.dma_start(
                out=iT_sb[:, :, j * SC:(j + 1) * SC],
                in_=iT_r[:, :, j * SC:(j + 1) * SC])
            nc.sync.dma_start(
                out=ih_sb[:, :, j * SC:(j + 1) * SC],
                in_=ih_r[:, :, j * SC:(j + 1) * SC])

        id16 = singles.tile([P, P], f16)
        from concourse.masks import make_identity
        make_identity(nc, id16)

        # f16x2 score operands. KA row 64 = ones, QA row 64 = -(m+32).
        KA = singles.tile([HDM + 1, LN], f16)      # [Khi; 1]
        KB = singles.tile([P, LN], f16)            # [Klo; Khi]
        QA = singles.tile([HDM + 1, NT * P], f16)  # [Qhi; -(m+32)]
        QB = singles.tile([P, NT * P], f16)        # [Qhi; Qlo]
        nc.vector.memset(KA[HDM:HDM + 1, :], 1.0)
        pv_sb = singles.tile([P, NKT, HDM + 1], f16)
        nc.vector.memset(pv_sb[:, :, HDM:HDM + 1], 1.0)

        sb = ctx.enter_context(tc.tile_pool(name="sbwork", bufs=3))
        stat = ctx.enter_context(tc.tile_pool(name="stat", bufs=2))
        # PSUM bank map (8 banks): a/qk(2) pv/pvblk(1) b(2) o(2) tiny(1)
        ppool = ctx.enter_context(tc.tile_pool(name="ppool", bufs=2, space="PSUM"))
        bpool = ctx.enter_context(tc.tile_pool(name="bpool", bufs=2, space="PSUM"))
        opool = ctx.enter_context(tc.tile_pool(name="opool", bufs=2, space="PSUM"))
        tiny = ctx.enter_context(tc.tile_pool(name="tiny", bufs=1, space="PSUM"))
        epool = ctx.enter_context(tc.tile_pool(name="epool", bufs=56))

        def pass_a(t):
            """Row max of tile t (bf16-hi scores) -> -m into QA row 64.
            Causal masking via PE additive mask blocks (maskd[2]=row-major
            tri, maskd[1]=partner); plain DVE reduce_max into mxp columns."""
            fc = t // 2
            dw = 256 if t % 2 == 0 else 512
            off = dw - 256
            lhsT = QA[0:HDM, t * P:(t + 1) * P]
            mxp = stat.tile([P, 10], f32, tag="mxp", name="mxp")
            for c in range(fc):
                aps = ppool.tile([P, SC], f32, tag="a", name="aps")
                nc.tensor.matmul(
                    aps, lhsT=lhsT, rhs=KA[0:HDM, c * SC:(c + 1) * SC],
                    start=True, stop=True)
                nc.vector.reduce_max(
                    out=mxp[:, c:c + 1], in_=aps, axis=mybir.AxisListType.X)
            dps = ppool.tile([P, SC], f32, tag="a", name="dps")
            nc.tensor.matmul(
                dps[:, 0:dw], lhsT=lhsT,
                rhs=KA[0:HDM, fc * SC:fc * SC + dw], start=True, stop=False)
            nc.tensor.matmul(
                dps[:, off:off + P], lhsT=id16, rhs=maskd_sb[:, 2, :],
                start=False, stop=False)
            nc.tensor.matmul(
                dps[:, off + P:off + 2 * P], lhsT=id16, rhs=maskd_sb[:, 1, :],
                start=False, stop=True)
            nc.vector.reduce_max(
                out=mxp[:, fc:fc + 1], in_=dps[:, 0:dw],
                axis=mybir.AxisListType.X)
            m = stat.tile([P, 1], f32, tag="m", name="m")
            nc.vector.reduce_max(
                out=m, in_=mxp[:, 0:fc + 1], axis=mybir.AxisListType.X)
            mneg = stat.tile([P, 1], f16, tag="mneg", name="mneg")
            nc.scalar.activation(
                out=mneg, in_=m, func=mybir.ActivationFunctionType.Copy,
                scale=-1.0, bias=-32.0)
            mt = tiny.tile([1, P], f16, tag="tp", name="mt")
            nc.tensor.transpose(mt, mneg, id16)
            nc.scalar.copy(out=QA[HDM:HDM + 1, t * P:(t + 1) * P], in_=mt)

        def b_work(gp, kt_lo, kt_hi):
            """bf16x2 transposed scores + exp for group gp (tiles
            4gp..4gp+3), then per-tile O accumulation from saved E tiles
            (each tile's O owns its psum bank for the whole group)."""
            tstart = 4 * gp
            q0 = tstart * P
            gw = 4 * P
            es = []
            for kt in range(kt_lo, kt_hi):
                t0l = max(0, kt // 2 - tstart)  # first covered group tile
                co = t0l * P
                kc = slice(kt * P, (kt + 1) * P)
                qc = slice(q0 + co, q0 + gw)
                bps = bpool.tile([P, SC], f32, tag="b", name="bps")
                in_band = kt >= 2 * tstart
                nc.tensor.matmul(
                    bps[:, co:gw], lhsT=KA[:, kc], rhs=QA[:, qc],
                    start=True, stop=False)
                nc.tensor.matmul(
                    bps[:, co:gw], lhsT=KB[:, kc], rhs=QB[:, qc],
                    start=False, stop=not in_band)
                if in_band:                    # diag tri / partner mask add
                    tl = kt // 2 - tstart
                    mi = kt % 2                # 0: tri mask, 1: partner mask
                    nc.tensor.matmul(
                        bps[:, tl * P:(tl + 1) * P], lhsT=id16,
                        rhs=maskd_sb[:, mi, :], start=False, stop=True)
                e_sb = epool.tile([P, SC], f16, tag="e", name="e_sb")
                nc.scalar.activation(
                    out=e_sb[:, co:gw], in_=bps[:, co:gw],
                    func=mybir.ActivationFunctionType.Exp, scale=0.125)
                es.append((kt, t0l, e_sb))
            for tl in range(4):
                t_abs = tstart + tl
                o_ps = opool.tile([P, HDM + 1], f32, tag="o", name="o_ps")
                last = 2 * t_abs + 1
                for kt, t0l, e_sb in es:
                    if tl < t0l or kt > last:
                        continue
                    nc.tensor.matmul(
                        o_ps, lhsT=e_sb[:, tl * P:(tl + 1) * P],
                        rhs=pv_sb[:, kt, :],
                        start=(kt == 0), stop=(kt == last))
                # finalize tile t_abs
                linv = stat.tile([P, 1], f32, tag="linv", name="linv")
                nc.vector.reciprocal(linv, o_ps[:, HDM:HDM + 1])
                on_sb = sb.tile([P, HDM], f16, tag="on", name="on_sb")
                nc.vector.tensor_scalar_mul(on_sb, o_ps[:, 0:HDM], linv)
                ot_ps = tiny.tile([HDM, P], f16, tag="tp", name="ot_ps")
                nc.tensor.transpose(ot_ps, on_sb, id16)
                ot_sb = sb.tile([HDM, P], f16, tag="otsb", name="ot_sb")
                nc.vector.tensor_copy(ot_sb, ot_ps)
                f_ps = bpool.tile([P, IDM], f32, tag="b", name="f_ps")
                nc.tensor.matmul(
                    f_ps, lhsT=ot_sb, rhs=v2_sb, start=True, stop=True)
                f_sb = sb.tile([P, IDM], f32, tag="fsb", name="f_sb")
                nc.vector.tensor_copy(f_sb, f_ps)
                nc.sync.dma_start(out=out[t_abs], in_=f_sb)

        # ---- main loop: fp32 proj slice c -> bf16x2 extraction -> pass A
        for c in range(LN // SC):
            sl = slice(c * SC, (c + 1) * SC)
            ps = ppool.tile([P, SC], f32, tag="a", name="ps")
            for d in range(4):
                nc.tensor.matmul(
                    ps, lhsT=wqk_sb[:, d, :], rhs=iT_sb[:, d, sl],
                    start=(d == 0), stop=(d == 3))
            # K extraction: hi (ACT) -> lo = ps - hi (DVE) -> hi replica (DVE)
            nc.scalar.copy(out=KA[0:HDM, sl], in_=ps[HDM:P, :])
            nc.vector.tensor_tensor(
                out=KB[0:HDM, sl], in0=ps[HDM:P, :], in1=KA[0:HDM, sl],
                op=mybir.AluOpType.subtract)
            nc.vector.tensor_copy(KB[HDM:P, sl], KA[0:HDM, sl])
            # Q extraction for own tiles 2c, 2c+1 (even psum block positions)
            for u in range(2):
                t = 2 * c + u
                qsl = slice(t * P, (t + 1) * P)
                psl = slice(2 * u * P, (2 * u + 1) * P)
                nc.scalar.copy(out=QA[0:HDM, qsl], in_=ps[0:HDM, psl])
                nc.vector.tensor_tensor(
                    out=QB[HDM:P, qsl], in0=ps[0:HDM, psl],
                    in1=QA[0:HDM, qsl], op=mybir.AluOpType.subtract)
                nc.vector.tensor_copy(QB[0:HDM, qsl], QA[0:HDM, qsl])
            # pv projection direct [key, hdm] layout from bf16 i
            for u in range(4):
                kt = 4 * c + u
                psv = ppool.tile([P, HDM], f32, tag="pv", bufs=1, name="psv")
                for d in range(4):
                    nc.tensor.matmul(
                        psv, lhsT=ih_sb[:, d, kt * P:(kt + 1) * P],
                        rhs=v1_sb[:, d, :], start=(d == 0), stop=(d == 3))
                nc.scalar.copy(out=pv_sb[:, kt, 0:HDM], in_=psv)
            pass_a(2 * c)
            pass_a(2 * c + 1)
            # B group gp woven in as soon as its row maxes complete
            if c % 2 == 1:
                gp = (c - 1) // 2
                b_work(gp, 0, 8 * gp + 8)

    nc.finalize()
    return nc


def make_core_inputs_v3(inputs):
    i = np.asarray(inputs["i"], dtype=np.float32)
    q = np.asarray(inputs["q"], dtype=np.float32)
    k = np.asarray(inputs["k"], dtype=np.float32)
    v1 = np.asarray(inputs["v1"], dtype=np.float32)
    v2 = np.asarray(inputs["v2"], dtype=np.float32)
    import ml_dtypes
    v2h = np.ascontiguousarray(v2.astype(np.float16))
    v1b = np.ascontiguousarray(v1.astype(ml_dtypes.bfloat16))
    wqk = np.ascontiguousarray(np.concatenate([q, k], axis=1))
    iota = np.arange(P, dtype=np.float32)
    # tri mask for S^T diag block: valid iff k_local <= q_local
    tri = np.where(iota[:, None] <= iota[None, :], 0.0, MASK_NEG).astype(np.float16)
    # row-major tri for pass A: valid iff k_local <= q_local (q on partitions)
    tri_r = np.where(iota[None, :] <= iota[:, None], 0.0, MASK_NEG).astype(np.float16)
    in_maps = []
    for core in range(8):
        b, h = core // 2, core % 2
        perm_blocks = []
        for j in range(NT):
            perm_blocks += [2 * j + h, 2 * j + 1 - h]
        cols = np.concatenate(
            [np.arange(P * g, P * g + P) for g in perm_blocks])
        iTp = np.ascontiguousarray(i[b].T[:, cols])      # [512, 4096]
        maskd = np.stack([
            tri,
            np.full((P, P), 0.0 if h == 1 else MASK_NEG, dtype=np.float16),
            tri_r,
        ]).astype(np.float16)
        import ml_dtypes
        in_maps.append({
            "iT": iTp, "wqk": wqk, "ih": iTp.astype(ml_dtypes.bfloat16),
            "v1s": v1b, "v2h": v2h,
            "maskd": np.ascontiguousarray(maskd),
        })
    return in_maps


_NC_CACHE = {}


def run_v3(inputs, **spmd_kwargs):
    if "v3" not in _NC_CACHE:
        _NC_CACHE["v3"] = build_nc_v3()
    nc = _NC_CACHE["v3"]
    in_maps = make_core_inputs_v3(inputs)
    res = run_bass_kernel_spmd(nc, in_maps, core_ids=list(range(8)), **spmd_kwargs)
    full = np.zeros((B, LN, IDM), dtype=np.float32)
    for core in range(8):
        b, h = core // 2, core % 2
        o = res.results[core]["out"]                 # [16, 128, 512] f32
        for t in range(NT):
            g = 2 * t + h
            full[b, g * P:(g + 1) * P] = o[t]
    return full, res


def kernel(**inputs):
    full, _ = run_v3(inputs)
    return full


# revision 26
# speedup vs baseline: 1.4933x; 1.0401x over previous
"""Trainium2 Bass kernel: single-head causal attention with low-rank V.

Math (per batch b):
  Q = i@q, K = i@k                  [4096, 64]
  S = Q K^T  (causal mask, /8)      [4096, 4096]
  A = softmax(S)
  out = A @ ((i@v1) @ v2) = ((A @ (i@v1)) / l) @ v2   (low-rank reassociation)

Sharding: 8 cores = 4 batches x 2 halves. Core half h owns query tiles
g = 2t+h (t=0..15). One SPMD program; per-core differences are data only
(column-permuted i^T so own tiles sit at even block positions, plus two
[128,128] mask blocks and a [128,2] window tensor).

v3 design (all matmuls fp32r at >=256 free / f16):
  P1: packed QK^T projection (wq|wk), pv = (i@v1) via pvT + PE transpose
  P2 (pass A, row-major): S chunks -> row max m via DVE tensor_mask_reduce
      (causal windows are data), -m written into row 64 of Q^T
  P3 (pass B, transposed): S'^T[k,q] = [K^T;1]^T-style 65-row contraction
      = K^T q - m  (m folded into matmul), exp on ACT -> E^T f16 in SBUF
      directly the lhsT of O accumulation (no PE transposes of E, no DVE
      copy-backs), O[q,65] += E^T^T @ [pv|1] gives both O and l.
      Finalize: O/l -> f16 -> transpose -> @ v2 -> out.
"""

import sys

if "/opt/trn_rl_repo" not in sys.path:
    sys.path.insert(0, "/opt/trn_rl_repo")

from contextlib import ExitStack

import numpy as np

import concourse.bass as bass
import concourse.tile as tile
from concourse import bacc
from concourse import mybir
from concourse.bass_utils import run_bass_kernel_spmd

B, LN, IDM, HDM = 4, 4096, 512, 64
P = 128          # partitions / tile rows
NT = 16          # query tiles per core
NKT = 32         # key (pos-)tiles per batch
SC = 512         # key chunk (1 psum bank of fp32)
MASK_NEG = -60000.0  # "-inf" in f16-representable units (pre-scale)


def build_nc_v3():
    """v4: fp32 projections (exact Q,K); scores via compensated bf16x2
    matmuls (m1 = [Khi;1]^T[Qhi;-m], m2 = [Klo;Khi]^T[Qhi;Qlo]); pass A
    (row max) in plain bf16-hi; pv path in fp32r via bitcast."""
    f32 = mybir.dt.float32
    f16 = mybir.dt.float16
    bf16 = mybir.dt.bfloat16
    f32r = mybir.dt.float32r
    nc = bacc.Bacc()

    ih = nc.dram_tensor("ih", [IDM, LN], f16, kind="ExternalInput")
    il = nc.dram_tensor("il", [IDM, LN], f16, kind="ExternalInput")
    wh = nc.dram_tensor("wh", [IDM, P], f16, kind="ExternalInput")
    wl = nc.dram_tensor("wl", [IDM, P], f16, kind="ExternalInput")
    v1s = nc.dram_tensor("v1s", [IDM, HDM], f16, kind="ExternalInput")
    v2h = nc.dram_tensor("v2h", [HDM, IDM], f16, kind="ExternalInput")
    maskd = nc.dram_tensor("maskd", [3, P, P], f16, kind="ExternalInput")
    out = nc.dram_tensor("out", [NT, P, IDM], f32, kind="ExternalOutput")

    with tile.TileContext(nc) as tc, ExitStack() as ctx:
        singles = ctx.enter_context(tc.tile_pool(name="singles", bufs=1))

        # ---- small inputs first so projections can start with iT slice 0
        wh_sb = singles.tile([P, 4, P], f16)
        nc.sync.dma_start(out=wh_sb, in_=wh.rearrange("(c p) h -> p c h", p=P))
        wl_sb = singles.tile([P, 4, P], f16)
        nc.sync.dma_start(out=wl_sb, in_=wl.rearrange("(c p) h -> p c h", p=P))
        v1_sb = singles.tile([P, 4, HDM], f16)
        nc.sync.dma_start(out=v1_sb, in_=v1s.rearrange("(c p) h -> p c h", p=P))
        v2_sb = singles.tile([HDM, IDM], f16)
        nc.sync.dma_start(out=v2_sb, in_=v2h[:, :])
        maskd_sb = singles.tile([P, 3, P], f16)
        nc.sync.dma_start(out=maskd_sb, in_=maskd.rearrange("m p q -> p m q"))
        ih_sb = singles.tile([P, 4, LN], f16)
        ih_r = ih.rearrange("(c p) n -> p c n", p=P)
        il_sb = singles.tile([P, 4, LN], f16)
        il_r = il.rearrange("(c p) n -> p c n", p=P)
        for j in range(LN // SC):
            nc.sync.dma_start(
                out=ih_sb[:, :, j * SC:(j + 1) * SC],
                in_=ih_r[:, :, j * SC:(j + 1) * SC])
            nc.sync.dma_start(
                out=il_sb[:, :, j * SC:(j + 1) * SC],
                in_=il_r[:, :, j * SC:(j + 1) * SC])

        id16 = singles.tile([P, P], f16)
        from concourse.masks import make_identity
        make_identity(nc, id16)

        # f16x2 score operands. KA row 64 = ones, QA row 64 = -(m+32).
        KA = singles.tile([HDM + 1, LN], f16)      # [Khi; 1]
        KB = singles.tile([P, LN], f16)            # [Klo; Khi]
        QA = singles.tile([HDM + 1, NT * P], f16)  # [Qhi; -(m+32)]
        QB = singles.tile([P, NT * P], f16)        # [Qhi; Qlo]
        nc.vector.memset(KA[HDM:HDM + 1, :], 1.0)
        pv_sb = singles.tile([P, NKT, HDM + 1], f16)
        nc.vector.memset(pv_sb[:, :, HDM:HDM + 1], 1.0)

        sb = ctx.enter_context(tc.tile_pool(name="sbwork", bufs=3))
        stat = ctx.enter_context(tc.tile_pool(name="stat", bufs=2))
        # PSUM bank map (8 banks): a/qk(2) pv/pvblk(1) b(2) o(2) tiny(1)
        ppool = ctx.enter_context(tc.tile_pool(name="ppool", bufs=2, space="PSUM"))
        bpool = ctx.enter_context(tc.tile_pool(name="bpool", bufs=2, space="PSUM"))
        opool = ctx.enter_context(tc.tile_pool(name="opool", bufs=2, space="PSUM"))
        tiny = ctx.enter_context(tc.tile_pool(name="tiny", bufs=1, space="PSUM"))
        epool = ctx.enter_context(tc.tile_pool(name="epool", bufs=56))

        def pass_a(t):
            """Row max of tile t (bf16-hi scores) -> -m into QA row 64.
            Causal masking via PE additive mask blocks (maskd[2]=row-major
            tri, maskd[1]=partner); plain DVE reduce_max into mxp columns."""
            fc = t // 2
            dw = 256 if t % 2 == 0 else 512
            off = dw - 256
            lhsT = QA[0:HDM, t * P:(t + 1) * P]
            mxp = stat.tile([P, 10], f32, tag="mxp", name="mxp")
            for c in range(fc):
                aps = ppool.tile([P, SC], f32, tag="a", name="aps")
                nc.tensor.matmul(
                    aps, lhsT=lhsT, rhs=KA[0:HDM, c * SC:(c + 1) * SC],
                    start=True, stop=True)
                nc.vector.reduce_max(
                    out=mxp[:, c:c + 1], in_=aps, axis=mybir.AxisListType.X)
            dps = ppool.tile([P, SC], f32, tag="a", name="dps")
            nc.tensor.matmul(
                dps[:, 0:dw], lhsT=lhsT,
                rhs=KA[0:HDM, fc * SC:fc * SC + dw], start=True, stop=False)
            nc.tensor.matmul(
                dps[:, off:off + P], lhsT=id16, rhs=maskd_sb[:, 2, :],
                start=False, stop=False)
            nc.tensor.matmul(
                dps[:, off + P:off + 2 * P], lhsT=id16, rhs=maskd_sb[:, 1, :],
                start=False, stop=True)
            nc.vector.reduce_max(
                out=mxp[:, fc:fc + 1], in_=dps[:, 0:dw],
                axis=mybir.AxisListType.X)
            m = stat.tile([P, 1], f32, tag="m", name="m")
            nc.vector.reduce_max(
                out=m, in_=mxp[:, 0:fc + 1], axis=mybir.AxisListType.X)
            mneg = stat.tile([P, 1], f16, tag="mneg", name="mneg")
            nc.scalar.activation(
                out=mneg, in_=m, func=mybir.ActivationFunctionType.Copy,
                scale=-1.0, bias=-32.0)
            mt = tiny.tile([1, P], f16, tag="tp", name="mt")
            nc.tensor.transpose(mt, mneg, id16)
            nc.scalar.copy(out=QA[HDM:HDM + 1, t * P:(t + 1) * P], in_=mt)

        def b_work(gp, kt_lo, kt_hi):
            """bf16x2 transposed scores + exp for group gp (tiles
            4gp..4gp+3), then per-tile O accumulation from saved E tiles
            (each tile's O owns its psum bank for the whole group)."""
            tstart = 4 * gp
            q0 = tstart * P
            gw = 4 * P
            es = []
            for kt in range(kt_lo, kt_hi):
                t0l = max(0, kt // 2 - tstart)  # first covered group tile
                co = t0l * P
                kc = slice(kt * P, (kt + 1) * P)
                qc = slice(q0 + co, q0 + gw)
                bps = bpool.tile([P, SC], f32, tag="b", name="bps")
                in_band = kt >= 2 * tstart
                nc.tensor.matmul(
                    bps[:, co:gw], lhsT=KA[:, kc], rhs=QA[:, qc],
                    start=True, stop=False)
                nc.tensor.matmul(
                    bps[:, co:gw], lhsT=KB[:, kc], rhs=QB[:, qc],
                    start=False, stop=not in_band)
                if in_band:                    # diag tri / partner mask add
                    tl = kt // 2 - tstart
                    mi = kt % 2                # 0: tri mask, 1: partner mask
                    nc.tensor.matmul(
                        bps[:, tl * P:(tl + 1) * P], lhsT=id16,
                        rhs=maskd_sb[:, mi, :], start=False, stop=True)
                e_sb = epool.tile([P, SC], f16, tag="e", name="e_sb")
                nc.scalar.activation(
                    out=e_sb[:, co:gw], in_=bps[:, co:gw],
                    func=mybir.ActivationFunctionType.Exp, scale=0.125)
                es.append((kt, t0l, e_sb))
            for tl in range(4):
                t_abs = tstart + tl
                o_ps = opool.tile([P, HDM + 1], f32, tag="o", name="o_ps")
                last = 2 * t_abs + 1
                for kt, t0l, e_sb in es:
                    if tl < t0l or kt > last:
                        continue
                    nc.tensor.matmul(
                        o_ps, lhsT=e_sb[:, tl * P:(tl + 1) * P],
                        rhs=pv_sb[:, kt, :],
                        start=(kt == 0), stop=(kt == last))
                # finalize tile t_abs
                linv = stat.tile([P, 1], f32, tag="linv", name="linv")
                nc.vector.reciprocal(linv, o_ps[:, HDM:HDM + 1])
                on_sb = sb.tile([P, HDM], f16, tag="on", name="on_sb")
                nc.vector.tensor_scalar_mul(on_sb, o_ps[:, 0:HDM], linv)
                ot_ps = tiny.tile([HDM, P], f16, tag="tp", name="ot_ps")
                nc.tensor.transpose(ot_ps, on_sb, id16)
                ot_sb = sb.tile([HDM, P], f16, tag="otsb", name="ot_sb")
                nc.vector.tensor_copy(ot_sb, ot_ps)
                f_ps = bpool.tile([P, IDM], f32, tag="b", name="f_ps")
                nc.tensor.matmul(
                    f_ps, lhsT=ot_sb, rhs=v2_sb, start=True, stop=True)
                f_sb = sb.tile([P, IDM], f32, tag="fsb", name="f_sb")
                nc.vector.tensor_copy(f_sb, f_ps)
                nc.sync.dma_start(out=out[t_abs], in_=f_sb)

        # ---- main loop: fp32 proj slice c -> bf16x2 extraction -> pass A
        for c in range(LN // SC):
            sl = slice(c * SC, (c + 1) * SC)
            ps = ppool.tile([P, SC], f32, tag="a", name="ps")
            for d in range(4):   # 3-term compensated f16 projection
                nc.tensor.matmul(
                    ps, lhsT=wh_sb[:, d, :], rhs=ih_sb[:, d, sl],
                    start=(d == 0), stop=False)
                nc.tensor.matmul(
                    ps, lhsT=wh_sb[:, d, :], rhs=il_sb[:, d, sl],
                    start=False, stop=False)
                nc.tensor.matmul(
                    ps, lhsT=wl_sb[:, d, :], rhs=ih_sb[:, d, sl],
                    start=False, stop=(d == 3))
            # K extraction: hi (ACT) -> lo = ps - hi (DVE) -> hi replica (DVE)
            nc.scalar.copy(out=KA[0:HDM, sl], in_=ps[HDM:P, :])
            nc.vector.tensor_tensor(
                out=KB[0:HDM, sl], in0=ps[HDM:P, :], in1=KA[0:HDM, sl],
                op=mybir.AluOpType.subtract)
            nc.vector.tensor_copy(KB[HDM:P, sl], KA[0:HDM, sl])
            # Q extraction for own tiles 2c, 2c+1 (even psum block positions)
            for u in range(2):
                t = 2 * c + u
                qsl = slice(t * P, (t + 1) * P)
                psl = slice(2 * u * P, (2 * u + 1) * P)
                nc.scalar.copy(out=QA[0:HDM, qsl], in_=ps[0:HDM, psl])
                nc.vector.tensor_tensor(
                    out=QB[HDM:P, qsl], in0=ps[0:HDM, psl],
                    in1=QA[0:HDM, qsl], op=mybir.AluOpType.subtract)
                nc.vector.tensor_copy(QB[0:HDM, qsl], QA[0:HDM, qsl])
            # pv projection direct [key, hdm] layout from bf16 i
            for u in range(4):
                kt = 4 * c + u
                psv = ppool.tile([P, HDM], f32, tag="pv", bufs=1, name="psv")
                for d in range(4):
                    nc.tensor.matmul(
                        psv, lhsT=ih_sb[:, d, kt * P:(kt + 1) * P],
                        rhs=v1_sb[:, d, :], start=(d == 0), stop=(d == 3))
                nc.scalar.copy(out=pv_sb[:, kt, 0:HDM], in_=psv)
            pass_a(2 * c)
            pass_a(2 * c + 1)
            # B group gp woven in as soon as its row maxes complete
            if c % 2 == 1:
                gp = (c - 1) // 2
                b_work(gp, 0, 8 * gp + 8)

    nc.finalize()
    return nc


def make_core_inputs_v3(inputs):
    i = np.asarray(inputs["i"], dtype=np.float32)
    q = np.asarray(inputs["q"], dtype=np.float32)
    k = np.asarray(inputs["k"], dtype=np.float32)
    v1 = np.asarray(inputs["v1"], dtype=np.float32)
    v2 = np.asarray(inputs["v2"], dtype=np.float32)
    v2h = np.ascontiguousarray(v2.astype(np.float16))
    v1b = np.ascontiguousarray(v1.astype(np.float16))
    wqk = np.concatenate([q, k], axis=1)
    wh = wqk.astype(np.float16)
    wl = (wqk - wh.astype(np.float32)).astype(np.float16)
    iota = np.arange(P, dtype=np.float32)
    # tri mask for S^T diag block: valid iff k_local <= q_local
    tri = np.where(iota[:, None] <= iota[None, :], 0.0, MASK_NEG).astype(np.float16)
    # row-major tri for pass A: valid iff k_local <= q_local (q on partitions)
    tri_r = np.where(iota[None, :] <= iota[:, None], 0.0, MASK_NEG).astype(np.float16)
    in_maps = []
    for core in range(8):
        b, h = core // 2, core % 2
        perm_blocks = []
        for j in range(NT):
            perm_blocks += [2 * j + h, 2 * j + 1 - h]
        cols = np.concatenate(
            [np.arange(P * g, P * g + P) for g in perm_blocks])
        iTp = np.ascontiguousarray(i[b].T[:, cols])      # [512, 4096]
        maskd = np.stack([
            tri,
            np.full((P, P), 0.0 if h == 1 else MASK_NEG, dtype=np.float16),
            tri_r,
        ]).astype(np.float16)
        ihp = iTp.astype(np.float16)
        ilp = (iTp - ihp.astype(np.float32)).astype(np.float16)
        in_maps.append({
            "ih": np.ascontiguousarray(ihp), "il": np.ascontiguousarray(ilp),
            "wh": np.ascontiguousarray(wh), "wl": np.ascontiguousarray(wl),
            "v1s": v1b, "v2h": v2h,
            "maskd": np.ascontiguousarray(maskd),
        })
    return in_maps


_NC_CACHE = {}


def run_v3(inputs, **spmd_kwargs):
    if "v3" not in _NC_CACHE:
        _NC_CACHE["v3"] = build_nc_v3()
    nc = _NC_CACHE["v3"]
    in_maps = make_core_inputs_v3(inputs)
    res = run_bass_kernel_spmd(nc, in_maps, core_ids=list(range(8)), **spmd_kwargs)
    full = np.zeros((B, LN, IDM), dtype=np.float32)
    for core in range(8):
        b, h = core // 2, core % 2
        o = res.results[core]["out"]                 # [16, 128, 512] f32
        for t in range(NT):
            g = 2 * t + h
            full[b, g * P:(g + 1) * P] = o[t]
    return full, res


def kernel(**inputs):
    full, _ = run_v3(inputs)
    return full


# revision 28
# speedup vs baseline: 1.5112x; 1.0120x over previous
"""Trainium2 Bass kernel: single-head causal attention with low-rank V.

Math (per batch b):
  Q = i@q, K = i@k                  [4096, 64]
  S = Q K^T  (causal mask, /8)      [4096, 4096]
  A = softmax(S)
  out = A @ ((i@v1) @ v2) = ((A @ (i@v1)) / l) @ v2   (low-rank reassociation)

Sharding: 8 cores = 4 batches x 2 halves. Core half h owns query tiles
g = 2t+h (t=0..15). One SPMD program; per-core differences are data only
(column-permuted i^T so own tiles sit at even block positions, plus two
[128,128] mask blocks and a [128,2] window tensor).

v3 design (all matmuls fp32r at >=256 free / f16):
  P1: packed QK^T projection (wq|wk), pv = (i@v1) via pvT + PE transpose
  P2 (pass A, row-major): S chunks -> row max m via DVE tensor_mask_reduce
      (causal windows are data), -m written into row 64 of Q^T
  P3 (pass B, transposed): S'^T[k,q] = [K^T;1]^T-style 65-row contraction
      = K^T q - m  (m folded into matmul), exp on ACT -> E^T f16 in SBUF
      directly the lhsT of O accumulation (no PE transposes of E, no DVE
      copy-backs), O[q,65] += E^T^T @ [pv|1] gives both O and l.
      Finalize: O/l -> f16 -> transpose -> @ v2 -> out.
"""

import sys

if "/opt/trn_rl_repo" not in sys.path:
    sys.path.insert(0, "/opt/trn_rl_repo")

from contextlib import ExitStack

import numpy as np

import concourse.bass as bass
import concourse.tile as tile
from concourse import bacc
from concourse import mybir
from concourse.bass_utils import run_bass_kernel_spmd

B, LN, IDM, HDM = 4, 4096, 512, 64
P = 128          # partitions / tile rows
NT = 16          # query tiles per core
NKT = 32         # key (pos-)tiles per batch
SC = 512         # key chunk (1 psum bank of fp32)
MASK_NEG = -60000.0  # "-inf" in f16-representable units (pre-scale)


def build_nc_v3():
    """v4: fp32 projections (exact Q,K); scores via compensated bf16x2
    matmuls (m1 = [Khi;1]^T[Qhi;-m], m2 = [Klo;Khi]^T[Qhi;Qlo]); pass A
    (row max) in plain bf16-hi; pv path in fp32r via bitcast."""
    f32 = mybir.dt.float32
    f16 = mybir.dt.float16
    bf16 = mybir.dt.bfloat16
    f32r = mybir.dt.float32r
    nc = bacc.Bacc()

    ih = nc.dram_tensor("ih", [IDM, LN], f16, kind="ExternalInput")
    il = nc.dram_tensor("il", [IDM, LN], f16, kind="ExternalInput")
    wh = nc.dram_tensor("wh", [IDM, P], f16, kind="ExternalInput")
    wl = nc.dram_tensor("wl", [IDM, P], f16, kind="ExternalInput")
    v1s = nc.dram_tensor("v1s", [IDM, HDM], f16, kind="ExternalInput")
    v2h = nc.dram_tensor("v2h", [HDM, IDM], f16, kind="ExternalInput")
    maskd = nc.dram_tensor("maskd", [3, P, P], f16, kind="ExternalInput")
    out = nc.dram_tensor("out", [NT, P, IDM], f32, kind="ExternalOutput")

    with tile.TileContext(nc) as tc, ExitStack() as ctx:
        singles = ctx.enter_context(tc.tile_pool(name="singles", bufs=1))

        # ---- small inputs first so projections can start with iT slice 0
        wh_sb = singles.tile([P, 4, P], f16)
        nc.sync.dma_start(out=wh_sb, in_=wh.rearrange("(c p) h -> p c h", p=P))
        wl_sb = singles.tile([P, 4, P], f16)
        nc.sync.dma_start(out=wl_sb, in_=wl.rearrange("(c p) h -> p c h", p=P))
        v1_sb = singles.tile([P, 4, HDM], f16)
        nc.sync.dma_start(out=v1_sb, in_=v1s.rearrange("(c p) h -> p c h", p=P))
        v2_sb = singles.tile([HDM, IDM], f16)
        nc.sync.dma_start(out=v2_sb, in_=v2h[:, :])
        maskd_sb = singles.tile([P, 3, P], f16)
        nc.sync.dma_start(out=maskd_sb, in_=maskd.rearrange("m p q -> p m q"))
        ih_sb = singles.tile([P, 4, LN], f16)
        ih_r = ih.rearrange("(c p) n -> p c n", p=P)
        il_sb = singles.tile([P, 4, LN], f16)
        il_r = il.rearrange("(c p) n -> p c n", p=P)
        for j in range(LN // SC):
            nc.sync.dma_start(
                out=ih_sb[:, :, j * SC:(j + 1) * SC],
                in_=ih_r[:, :, j * SC:(j + 1) * SC])
            nc.sync.dma_start(
                out=il_sb[:, :, j * SC:(j + 1) * SC],
                in_=il_r[:, :, j * SC:(j + 1) * SC])

        id16 = singles.tile([P, P], f16)
        from concourse.masks import make_identity
        make_identity(nc, id16)

        # f16x2 score operands. KA row 64 = ones, QA row 64 = -(m+32).
        KA = singles.tile([HDM + 1, LN], f16)      # [Khi; 1]
        KB = singles.tile([P, LN], f16)            # [Klo; Khi]
        QA = singles.tile([HDM + 1, NT * P], f16)  # [Qhi; -(m+32)]
        QB = singles.tile([P, NT * P], f16)        # [Qhi; Qlo]
        nc.vector.memset(KA[HDM:HDM + 1, :], 1.0)
        pv_sb = singles.tile([P, NKT, HDM + 1], f16)
        nc.vector.memset(pv_sb[:, :, HDM:HDM + 1], 1.0)

        sb = ctx.enter_context(tc.tile_pool(name="sbwork", bufs=3))
        stat = ctx.enter_context(tc.tile_pool(name="stat", bufs=2))
        # PSUM bank map (8 banks): a/qk(2) pv/pvblk(1) b(2) o(2) tiny(1)
        ppool = ctx.enter_context(tc.tile_pool(name="ppool", bufs=2, space="PSUM"))
        bpool = ctx.enter_context(tc.tile_pool(name="bpool", bufs=2, space="PSUM"))
        opool = ctx.enter_context(tc.tile_pool(name="opool", bufs=2, space="PSUM"))
        tiny = ctx.enter_context(tc.tile_pool(name="tiny", bufs=1, space="PSUM"))
        epool = ctx.enter_context(tc.tile_pool(name="epool", bufs=56))

        def pass_a(t):
            """Row max of tile t (bf16-hi scores) -> -m into QA row 64.
            Causal masking via PE additive mask blocks (maskd[2]=row-major
            tri, maskd[1]=partner); plain DVE reduce_max into mxp columns."""
            fc = t // 2
            dw = 256 if t % 2 == 0 else 512
            off = dw - 256
            lhsT = QA[0:HDM, t * P:(t + 1) * P]
            mxp = stat.tile([P, 10], f32, tag="mxp", name="mxp")
            for c in range(fc):
                aps = ppool.tile([P, SC], f32, tag="a", name="aps")
                nc.tensor.matmul(
                    aps, lhsT=lhsT, rhs=KA[0:HDM, c * SC:(c + 1) * SC],
                    start=True, stop=True)
                nc.vector.reduce_max(
                    out=mxp[:, c:c + 1], in_=aps, axis=mybir.AxisListType.X)
            dps = ppool.tile([P, SC], f32, tag="a", name="dps")
            nc.tensor.matmul(
                dps[:, 0:dw], lhsT=lhsT,
                rhs=KA[0:HDM, fc * SC:fc * SC + dw], start=True, stop=False)
            nc.tensor.matmul(
                dps[:, off:off + P], lhsT=id16, rhs=maskd_sb[:, 2, :],
                start=False, stop=False)
            nc.tensor.matmul(
                dps[:, off + P:off + 2 * P], lhsT=id16, rhs=maskd_sb[:, 1, :],
                start=False, stop=True)
            nc.vector.reduce_max(
                out=mxp[:, fc:fc + 1], in_=dps[:, 0:dw],
                axis=mybir.AxisListType.X)
            m = stat.tile([P, 1], f32, tag="m", name="m")
            nc.vector.reduce_max(
                out=m, in_=mxp[:, 0:fc + 1], axis=mybir.AxisListType.X)
            mneg = stat.tile([P, 1], f16, tag="mneg", name="mneg")
            nc.scalar.activation(
                out=mneg, in_=m, func=mybir.ActivationFunctionType.Copy,
                scale=-1.0, bias=-32.0)
            mt = tiny.tile([1, P], f16, tag="tp", name="mt")
            nc.tensor.transpose(mt, mneg, id16)
            nc.scalar.copy(out=QA[HDM:HDM + 1, t * P:(t + 1) * P], in_=mt)

        def b_work(gp, kt_lo, kt_hi):
            """bf16x2 transposed scores + exp for group gp (tiles
            4gp..4gp+3), then per-tile O accumulation from saved E tiles
            (each tile's O owns its psum bank for the whole group)."""
            tstart = 4 * gp
            q0 = tstart * P
            gw = 4 * P
            es = []
            for kt in range(kt_lo, kt_hi):
                t0l = max(0, kt // 2 - tstart)  # first covered group tile
                co = t0l * P
                kc = slice(kt * P, (kt + 1) * P)
                qc = slice(q0 + co, q0 + gw)
                bps = bpool.tile([P, SC], f32, tag="b", name="bps")
                in_band = kt >= 2 * tstart
                nc.tensor.matmul(
                    bps[:, co:gw], lhsT=KA[:, kc], rhs=QA[:, qc],
                    start=True, stop=False)
                nc.tensor.matmul(
                    bps[:, co:gw], lhsT=KB[:, kc], rhs=QB[:, qc],
                    start=False, stop=not in_band)
                if in_band:                    # diag tri / partner mask add
                    tl = kt // 2 - tstart
                    mi = kt % 2                # 0: tri mask, 1: partner mask
                    nc.tensor.matmul(
                        bps[:, tl * P:(tl + 1) * P], lhsT=id16,
                        rhs=maskd_sb[:, mi, :], start=False, stop=True)
                e_sb = epool.tile([P, SC], f16, tag="e", name="e_sb")
                nc.scalar.activation(
                    out=e_sb[:, co:gw], in_=bps[:, co:gw],
                    func=mybir.ActivationFunctionType.Exp, scale=0.125)
                es.append((kt, t0l, e_sb))
            for tl in range(4):
                t_abs = tstart + tl
                o_ps = opool.tile([P, HDM + 1], f32, tag="o", name="o_ps")
                last = 2 * t_abs + 1
                for kt, t0l, e_sb in es:
                    if tl < t0l or kt > last:
                        continue
                    nc.tensor.matmul(
                        o_ps, lhsT=e_sb[:, tl * P:(tl + 1) * P],
                        rhs=pv_sb[:, kt, :],
                        start=(kt == 0), stop=(kt == last))
                # finalize tile t_abs
                linv = stat.tile([P, 1], f32, tag="linv", name="linv")
                nc.vector.reciprocal(linv, o_ps[:, HDM:HDM + 1])
                on_sb = sb.tile([P, HDM], f16, tag="on", name="on_sb")
                nc.vector.tensor_scalar_mul(on_sb, o_ps[:, 0:HDM], linv)
                ot_ps = tiny.tile([HDM, P], f16, tag="tp", name="ot_ps")
                nc.tensor.transpose(ot_ps, on_sb, id16)
                ot_sb = sb.tile([HDM, P], f16, tag="otsb", name="ot_sb")
                nc.vector.tensor_copy(ot_sb, ot_ps)
                f_ps = bpool.tile([P, IDM], f32, tag="b", name="f_ps")
                nc.tensor.matmul(
                    f_ps, lhsT=ot_sb, rhs=v2_sb, start=True, stop=True)
                f_sb = sb.tile([P, IDM], f32, tag="fsb", name="f_sb")
                nc.vector.tensor_copy(f_sb, f_ps)
                nc.sync.dma_start(out=out[t_abs], in_=f_sb)

        # ---- main loop: fp32 proj slice c -> bf16x2 extraction -> pass A
        for c in range(LN // SC):
            sl = slice(c * SC, (c + 1) * SC)
            ps = ppool.tile([P, SC], f32, tag="a", name="ps")
            for d in range(4):   # 3-term compensated f16 projection
                nc.tensor.matmul(
                    ps, lhsT=wh_sb[:, d, :], rhs=ih_sb[:, d, sl],
                    start=(d == 0), stop=False)
                nc.tensor.matmul(
                    ps, lhsT=wh_sb[:, d, :], rhs=il_sb[:, d, sl],
                    start=False, stop=False)
                nc.tensor.matmul(
                    ps, lhsT=wl_sb[:, d, :], rhs=ih_sb[:, d, sl],
                    start=False, stop=(d == 3))
            # K extraction: hi (ACT) -> lo = ps - hi (DVE) -> hi replica (DVE)
            nc.scalar.copy(out=KA[0:HDM, sl], in_=ps[HDM:P, :])
            nc.vector.tensor_tensor(
                out=KB[0:HDM, sl], in0=ps[HDM:P, :], in1=KA[0:HDM, sl],
                op=mybir.AluOpType.subtract)
            nc.vector.tensor_copy(KB[HDM:P, sl], KA[0:HDM, sl])
            # Q extraction for own tiles 2c, 2c+1 (even psum block positions)
            for u in range(2):
                t = 2 * c + u
                qsl = slice(t * P, (t + 1) * P)
                psl = slice(2 * u * P, (2 * u + 1) * P)
                nc.scalar.copy(out=QA[0:HDM, qsl], in_=ps[0:HDM, psl])
                nc.vector.tensor_tensor(
                    out=QB[HDM:P, qsl], in0=ps[0:HDM, psl],
                    in1=QA[0:HDM, qsl], op=mybir.AluOpType.subtract)
                nc.vector.tensor_copy(QB[0:HDM, qsl], QA[0:HDM, qsl])
            # pv projection direct [key, hdm] layout from bf16 i
            for u in range(4):
                kt = 4 * c + u
                psv = ppool.tile([P, HDM], f32, tag="pv", bufs=1, name="psv")
                for d in range(4):
                    nc.tensor.matmul(
                        psv, lhsT=ih_sb[:, d, kt * P:(kt + 1) * P],
                        rhs=v1_sb[:, d, :], start=(d == 0), stop=(d == 3))
                nc.scalar.copy(out=pv_sb[:, kt, 0:HDM], in_=psv)
            pass_a(2 * c)
            pass_a(2 * c + 1)
            # B group gp woven in as soon as its row maxes complete
            if c % 2 == 1:
                gp = (c - 1) // 2
                b_work(gp, 0, 8 * gp + 8)

    nc.finalize()
    return nc


def make_core_inputs_v3(inputs):
    i = np.asarray(inputs["i"], dtype=np.float32)
    q = np.asarray(inputs["q"], dtype=np.float32)
    k = np.asarray(inputs["k"], dtype=np.float32)
    v1 = np.asarray(inputs["v1"], dtype=np.float32)
    v2 = np.asarray(inputs["v2"], dtype=np.float32)
    v2h = np.ascontiguousarray(v2.astype(np.float16))
    v1b = np.ascontiguousarray(v1.astype(np.float16))
    wqk = np.concatenate([q, k], axis=1)
    wh = wqk.astype(np.float16)
    wl = (wqk - wh.astype(np.float32)).astype(np.float16)
    iota = np.arange(P, dtype=np.float32)
    # tri mask for S^T diag block: valid iff k_local <= q_local
    tri = np.where(iota[:, None] <= iota[None, :], 0.0, MASK_NEG).astype(np.float16)
    # row-major tri for pass A: valid iff k_local <= q_local (q on partitions)
    tri_r = np.where(iota[None, :] <= iota[:, None], 0.0, MASK_NEG).astype(np.float16)
    in_maps = []
    for core in range(8):
        b, h = core // 2, core % 2
        perm_blocks = []
        for j in range(NT):
            perm_blocks += [2 * j + h, 2 * j + 1 - h]
        cols = np.concatenate(
            [np.arange(P * g, P * g + P) for g in perm_blocks])
        iTp = np.ascontiguousarray(i[b].T[:, cols])      # [512, 4096]
        maskd = np.stack([
            tri,
            np.full((P, P), 0.0 if h == 1 else MASK_NEG, dtype=np.float16),
            tri_r,
        ]).astype(np.float16)
        ihp = iTp.astype(np.float16)
        ilp = (iTp - ihp.astype(np.float32)).astype(np.float16)
        in_maps.append({
            "ih": np.ascontiguousarray(ihp), "il": np.ascontiguousarray(ilp),
            "wh": np.ascontiguousarray(wh), "wl": np.ascontiguousarray(wl),
            "v1s": v1b, "v2h": v2h,
            "maskd": np.ascontiguousarray(maskd),
        })
    return in_maps


_NC_CACHE = {}


def run_v3(inputs, **spmd_kwargs):
    if "v3" not in _NC_CACHE:
        _NC_CACHE["v3"] = build_nc_v3()
    nc = _NC_CACHE["v3"]
    in_maps = make_core_inputs_v3(inputs)
    res = run_bass_kernel_spmd(nc, in_maps, core_ids=list(range(8)), **spmd_kwargs)
    full = np.zeros((B, LN, IDM), dtype=np.float32)
    for core in range(8):
        b, h = core // 2, core % 2
        o = res.results[core]["out"]                 # [16, 128, 512] f32
        for t in range(NT):
            g = 2 * t + h
            full[b, g * P:(g + 1) * P] = o[t]
    return full, res


def kernel(**inputs):
    full, _ = run_v3(inputs)
    return full
